# revision 7
# baseline (speedup 1.0000x reference)
"""Trainium2 Bass kernel for 8x8 block 2D-DCT (nn_DCT2d).

Input : x (32, 1, 1024, 1024) fp32
Output: coeff (32, 16384, 8, 8) fp32 where coeff[n,k] = A @ block_k @ A^T

Strategy (pure data parallel, 4 images per core across 8 cores):
  Per core, per (image n, row-chunk R of 128 rows):
   1. DMA-load T0[128,1024]: partition p = (bi0, i, bihi3) [bit-permuted
      row order], free = w. Each partition is one contiguous 4KB image row.
   2/3. Two DVE 32x32 stream-transposes with strided APs rearrange to
      X2'[(bi0, i, j), (bihi3, bj)]: a block-pair's 128 pixels on the
      partition dim, block index along free.
   4. One 128x128 matmul per 128 blocks with the constant blockdiag
      kron(A,A) weight: out[bj, (bi0,u,v)] -- both DCT passes at once.
   5. DMA-store with 256B-contiguous runs into the (n, k, u, v) layout.
"""
import numpy as np
from contextlib import ExitStack

import concourse.bass as bass
import concourse.tile as tile
from concourse import bacc, mybir
from concourse.bass_utils import run_bass_kernel_spmd

N_CORES = 8
IMGS_PER_CORE = 4  # 32 / 8
F32 = mybir.dt.float32

_BS = 8


def _make_dct_matrix(bs=_BS):
    A = np.zeros((bs, bs), dtype=np.float64)
    for i in range(bs):
        c_i = 1.0 / np.sqrt(2.0) if i == 0 else 1.0
        for n in range(bs):
            A[i, n] = np.sqrt(2.0 / bs) * c_i * np.cos((2 * n + 1) / (bs * 2) * i * np.pi)
    return A.astype(np.float32)


def _make_w2(A):
    """W2[k=(bi0,i,j), n=(bi0',u,v)] = delta(bi0==bi0') * A[u,i]*A[v,j]."""
    A = np.asarray(A, dtype=np.float32)
    kron = np.einsum("ui,vj->ijuv", A, A).reshape(64, 64)  # [(i,j), (u,v)]
    W2 = np.zeros((128, 128), dtype=np.float32)
    W2[:64, :64] = kron
    W2[64:, 64:] = kron
    return W2


def _stream_transpose(nc, out_ap, in_ap):
    """Raw InstStreamTranspose: flat (1-D mem pattern) source, scattered dest.

    Walrus requires the src pattern to be 1-D; the dst AP may scatter, which
    lets each transpose fuse an arbitrary free-dim permutation into its write.
    """
    eng = nc.vector
    return eng.add_instruction(
        mybir.InstStreamTranspose(
            name=nc.get_next_instruction_name(),
            ins=[eng.lower_ap(in_ap)],
            outs=[eng.lower_ap(out_ap)],
        )
    )


def build_nc(n_imgs=IMGS_PER_CORE):
    nc = bacc.Bacc(
        "TRN2",
        target_bir_lowering=False,
        debug=False,
        num_devices=N_CORES,
    )
    x = nc.dram_tensor("x", [n_imgs * 1024, 1024], F32, kind="ExternalInput")
    w2 = nc.dram_tensor("w2", [128, 128], F32, kind="ExternalInput")
    out = nc.dram_tensor("out", [n_imgs * 1048576], F32, kind="ExternalOutput")

    # row = ((((n*8 + R)*8 + h)*2 + b0)*8 + i) ; partition p = (b0, i, h)
    xv = x.ap().rearrange(
        "(n R h b0 i) w -> n R b0 i h w", n=n_imgs, R=8, h=8, b0=2, i=8
    )
    # out addr = n*2^20 + R*2^17 + H*2^16 + q*2^14 + b0*2^13 + bj*64 + uv
    ov = out.ap().rearrange(
        "(n R H q b0 bj uv) -> n R H bj q b0 uv",
        n=n_imgs, R=8, H=2, q=4, b0=2, bj=128, uv=64,
    )

    with tile.TileContext(nc) as tc, ExitStack() as ctx:
        w2p = ctx.enter_context(tc.tile_pool(name="w2", bufs=1))
        t0p = ctx.enter_context(tc.tile_pool(name="t0", bufs=3))
        i1p = ctx.enter_context(tc.tile_pool(name="i1", bufs=2))
        x2p = ctx.enter_context(tc.tile_pool(name="x2", bufs=2))
        x3p = ctx.enter_context(tc.tile_pool(name="x3", bufs=2))
        psp = ctx.enter_context(
            tc.tile_pool(name="ps", bufs=4, space=bass.MemorySpace.PSUM)
        )
        stp = ctx.enter_context(tc.tile_pool(name="st", bufs=4))

        w2t = w2p.tile([128, 128], F32)
        nc.sync.dma_start(w2t[:], w2.ap())

        for n in range(n_imgs):
            for R in range(8):
                t0 = t0p.tile([128, 1024], F32)
                nc.sync.dma_start(t0[:], xv[n, R])

                # T1: pull (bj1,bj0,j) onto p[4:0]; expel (i1,i0,bh) to free.
                # dst: logical (a,b) -> phys b*32 + a
                i1 = i1p.tile([128, 1024], F32)
                _stream_transpose(
                    nc, i1[:].rearrange("p (b a) -> p a b", b=32, a=32), t0[:]
                )
                # T2: pull (bj6..bj2); expel (bj1,bj0,j).
                # dst: logical (c1,c2,d1,d2) -> phys c2*128 + d1*32 + c1*8 + d2
                x2 = x2p.tile([128, 1024], F32)
                _stream_transpose(
                    nc,
                    x2[:].rearrange(
                        "p (c2 d1 c1 d2) -> p c1 c2 d1 d2", c2=8, d1=4, c1=4, d2=8
                    ),
                    i1[:],
                )
                # T3: pull (i1,i0,j) -> p' = (bi0, i, j); expel (bj6..bj2).
                # dst: logical (e1,e2,f) -> phys e1*128 + f*4 + e2  = (bihi3, bj)
                x3 = x3p.tile([128, 1024], F32)
                _stream_transpose(
                    nc,
                    x3[:].rearrange("p (e1 f e2) -> p e1 e2 f", e1=8, f=32, e2=4),
                    x2[:],
                )

                for H in range(2):
                    ps = psp.tile([128, 512], F32)
                    for q in range(4):
                        c = H * 4 + q
                        nc.tensor.matmul(
                            ps[:, q * 128:(q + 1) * 128],
                            x3[:, c * 128:(c + 1) * 128],
                            w2t[:],
                            start=True,
                            stop=True,
                        )
                    st = stp.tile([128, 512], F32)
                    nc.scalar.copy(st[:], ps[:])
                    nc.sync.dma_start(
                        ov[n, R, H],
                        st[:].rearrange("p (q b0 uv) -> p q b0 uv", q=4, b0=2, uv=64),
                    )

    nc.compile()
    return nc


_NC_CACHE = {}


def _get_nc():
    if "nc" not in _NC_CACHE:
        _NC_CACHE["nc"] = build_nc()
    return _NC_CACHE["nc"]


def kernel(x, A=None, **_ignored):
    x = np.ascontiguousarray(np.asarray(x, dtype=np.float32))
    assert x.shape == (32, 1, 1024, 1024), x.shape
    if A is None:
        A = _make_dct_matrix()
    w2 = _make_w2(A)

    nc = _get_nc()
    xf = x.reshape(32, 1024, 1024)
    in_maps = []
    for c in range(N_CORES):
        shard = xf[c * IMGS_PER_CORE:(c + 1) * IMGS_PER_CORE].reshape(
            IMGS_PER_CORE * 1024, 1024
        )
        in_maps.append({"x": np.ascontiguousarray(shard), "w2": w2})

    res = run_bass_kernel_spmd(nc, in_maps, list(range(N_CORES)))
    outs = [
        res.results[c]["out"].reshape(IMGS_PER_CORE, 16384, 8, 8)
        for c in range(N_CORES)
    ]
    return np.concatenate(outs, axis=0)
